# revision 65
# baseline (speedup 1.0000x reference)
"""Multi-head attention (B=8, S=1024, E=768, H=12) on 8 trn2 NeuronCores.

Strategy: batch-parallel — core b processes batch element b end-to-end, no
collectives.  All matmuls in bf16 with fp32 PSUM accumulation.  The host
pre-transposes x (and casts to bf16) and pre-tiles every operand into its
exact SBUF image [128, NE*width], so each load is one DMA of 128
contiguous descriptors — descriptor generation was the front bottleneck.

Per-core dataflow (token s/t, feature e, head h, head-dim d):
  xT[e, s]    loaded directly (host-transposed, bf16)
  q/k proj    psum[hd, s-chunk] = Wq/Wk-tile^T @ xT; DVE adds bias ->
              qt/kt bf16; emitted one pair ahead of its scores
  v[t, hdA]   xT^T @ WvT; col h*65+64 memset to ones -> softmax
              denominator rides the attn matmul; bv is folded into the
              output bias on the host (bo' = bo + Wo@bv, exact since the
              bv term is constant per token); emitted inside pair 0 so
              it fills the PE stalls while ACT runs pair-0's exps
  scoresT     kT_h^T @ qT_h per head (K=64; the two heads of a pair run
              on disjoint PE row halves via tile_position)
  expT        ACT exp with scale=1/8 (scores kept unscaled), bf16; ACT is
              the second-busiest engine so nothing else runs on it
              mid-kernel
  attn_aug    v^T @ expT accumulated over t in [65, 512] column-granule
              psum groups (row 64 = denominator); bufs=2 granule rotation
              overlaps consecutive heads' psum recycling
  normalize   den row -> DRAM -> [128, 8] (DVE reciprocal is serial in
              free size, so recip runs on the reshaped layout) -> DRAM ->
              partition-broadcast; the last pair interleaves heads per
              column half on two DMA queues, so the first half-columns'
              chains (and the output projection's k=5 matmuls) launch
              ~6us before the full attention completes
  out[s, f]   catT^T @ WoT, k=5 ordered last so the 4-deep PE parking
              window fills the last normalize latency with ready k<=4
              work; bias added during the DVE psum drain (bo arrives
              pre-broadcast from the host), DMA out
"""

import os
import numpy as np
import ml_dtypes

B, S, E, H, DH = 8, 1024, 768, 12, 64
EA = E + 1          # augmented contraction dim (ones/bias row)
HW = DH + 1         # per-head V width (d cols + ones col)
VW = H * HW         # 780
NT = S // 128       # 8 token tiles
NE = E // 128       # 6 feature tiles

_cache = {}


def _split_multiwaits(nc):
    """This toolchain's walrus encodes at most one sync-wait per instruction
    (two for EventSemaphore).  Tile's epilogue can attach more; hoist the
    extras onto same-engine NOPs placed immediately before the instruction —
    the engine sequencer executes in order, so semantics are unchanged."""
    import concourse.mybir as mybir

    for bb in nc.main_func.blocks:
        out, changed = [], False
        for ins in bb.instructions:
            si = ins.sync_info
            cap = 2 if isinstance(ins, mybir.InstEventSemaphore) else 1
            if si is not None and si.on_wait and len(si.on_wait) > cap:
                waits = list(si.on_wait)
                for w_i, w in enumerate(waits[:-cap]):
                    out.append(mybir.InstNoOp(
                        name=f"{ins.name}-wsplit{w_i}",
                        engine=ins.engine,
                        sync_info=mybir.SyncInfo(on_wait=[w], on_update=[]),
                        bass_nofuse=True,
                    ))
                ins.sync_info = mybir.SyncInfo(
                    on_wait=waits[-cap:], on_update=list(si.on_update))
                changed = True
            out.append(ins)
        if changed:
            bb.instructions = out


def _dedupe_ldweights(nc):
    """Delete an InstLdweights when the immediately-preceding PE-stream
    instructions are its identical twin followed only by plain (non-transpose)
    matmuls — the weights are still resident in the array.  Only waitless,
    updateless LDWs are removed."""
    import concourse.mybir as mybir

    ndel = 0
    for bb in nc.main_func.blocks:
        out = []
        prev_key = None          # signature of weights currently in the array
        changed = False
        for ins in bb.instructions:
            if isinstance(ins, mybir.InstLdweights):
                si = ins.sync_info
                clean = not si or (not si.on_wait and not si.on_update)
                key = (str(ins.ins[0]), str(ins.tile_position),
                       str(ins.perf_mode), str(ins.is_transpose))
                if clean and key == prev_key:
                    ndel += 1
                    changed = True
                    continue
                prev_key = key
            elif isinstance(ins, mybir.InstMatmult):
                if ins.is_transpose:
                    prev_key = None   # transpose streams data into the array
            elif ins.engine == mybir.EngineType.PE:
                prev_key = None
            out.append(ins)
        if changed:
            bb.instructions = out
    return ndel


def _patch_ldw_opt():
    """Flip walrus --enable-ldw-opt (hardcoded false in bass_utils) via a
    run_command shim."""
    import concourse.bass_utils as bu
    if getattr(bu, "_mha_ldw_patched", False):
        return
    orig = bu.run_command

    def run_command_ldw(argv, **kw):
        argv = ["--enable-ldw-opt=true" if a == "--enable-ldw-opt=false" else a
                for a in argv]
        return orig(argv, **kw)

    bu.run_command = run_command_ldw
    bu._mha_ldw_patched = True


def _build_bass(split_waits=True):
    import concourse.bass as bass
    import concourse.tile as tile
    import concourse.mybir as mybir
    from contextlib import ExitStack

    f32 = mybir.dt.float32
    bf16 = mybir.dt.bfloat16
    EXP = mybir.ActivationFunctionType.Exp

    nc = bass.Bass(trn_type="TRN2")

    # All inputs arrive host-pre-tiled as the exact SBUF image
    # [128, NE*width] so every load is 128 contiguous descriptors.
    xt_d = nc.dram_tensor("xtr", [128, NE * S], bf16, kind="ExternalInput")
    wqt_d = nc.dram_tensor("wqt", [128, NE * E], bf16, kind="ExternalInput")
    wkt_d = nc.dram_tensor("wkt", [128, NE * E], bf16, kind="ExternalInput")
    bq_d = nc.dram_tensor("bq", [128, NE], f32, kind="ExternalInput")
    bk_d = nc.dram_tensor("bk", [128, NE], f32, kind="ExternalInput")
    wvt_d = nc.dram_tensor("wvt", [128, NE * VW], bf16, kind="ExternalInput")
    wot_d = nc.dram_tensor("wot", [128, NE * E], bf16, kind="ExternalInput")
    bob_d = nc.dram_tensor("bob", [128, E], bf16, kind="ExternalInput")
    out_d = nc.dram_tensor("out", [S, E], f32, kind="ExternalOutput")

    with tile.TileContext(nc) as tc, ExitStack() as ctx:
        singles = ctx.enter_context(tc.tile_pool(name="singles", bufs=1))

        # (the augmented ones-row is gone: both bias folds removed its users)
        ones_row = None

        # ---- input DMAs.  Big transfers serialize on the SP queue in
        # priority order (descriptor-gen of DMA n+1 overlaps transfer n);
        # the two tiny bias loads go to the Pool queue. ----
        xt_all = singles.tile([128, NE * S], bf16, tag="xta", name="xtall")
        nc.sync.dma_start(out=xt_all, in_=xt_d[:, :])
        xt = [xt_all[:, j * S:(j + 1) * S] for j in range(NE)]

        # biases as one [128, NE] tile each (column m = k-tile m)
        bq_sb = singles.tile([128, NE], f32, tag="bqs", name="bqs")
        nc.gpsimd.dma_start(out=bq_sb, in_=bq_d[:, :])
        bk_sb = singles.tile([128, NE], f32, tag="bks", name="bks")
        nc.gpsimd.dma_start(out=bk_sb, in_=bk_d[:, :])
        bqs = [bq_sb[:, m:m + 1] for m in range(NE)]
        bks = [bk_sb[:, m:m + 1] for m in range(NE)]

        class WView:
            """All k-tiles of a weight in one SBUF tile (one DMA)."""
            def __init__(self, all_tile, width, bias_tile):
                self.all, self.width, self.bias = all_tile, width, bias_tile

            def __getitem__(self, k):
                if self.bias is not None and k == NE:
                    return self.bias
                return _WSlice(self, k)

        class _WSlice:
            def __init__(self, v, k):
                self.v, self.k = v, k

            def __getitem__(self, idx):
                _, cols = idx
                off = self.k * self.v.width
                return self.v.all[:, off + cols.start:off + cols.stop]

        def load_w(dram, width, bias_dram, halves=1):
            t = singles.tile([128, NE * width], bf16, tag=f"w{dram.name}",
                             name=f"w{dram.name}")
            hw_ = NE * width // halves
            for h in range(halves):
                nc.sync.dma_start(out=t[:, h * hw_:(h + 1) * hw_],
                                  in_=dram[:, h * hw_:(h + 1) * hw_])
            bias_t = None
            if bias_dram is not None:
                bias_t = singles.tile([1, width], bf16, tag=f"w{dram.name}b",
                                      name=f"w{dram.name}b")
                nc.gpsimd.dma_start(out=bias_t, in_=bias_dram[:, :])
            return WView(t, width, bias_t)

        wq = load_w(wqt_d, E, None, halves=2)
        wk = load_w(wkt_d, E, None, halves=2)
        wv = load_w(wvt_d, VW, None)
        wo = load_w(wot_d, E, None)
        bo_bc = singles.tile([128, E], bf16, tag="bob", name="bob")
        nc.gpsimd.dma_start(out=bo_bc, in_=bob_d[:, :])

        def xa(k):  # augmented xT rows
            return xt[k] if k < NE else ones_row

        # ---- steady-state tiles ----
        vt = [singles.tile([128, VW], bf16, tag=f"vt{i}", name=f"vt{i}")
              for i in range(NT)]
        catt = [singles.tile([128, S], bf16, tag=f"ct{j}", name=f"ct{j}")
                for j in range(NE)]

        with ExitStack() as sctx:
            qk8p = sctx.enter_context(tc.tile_pool(name="qk8", bufs=2))
            expp = sctx.enter_context(tc.tile_pool(name="exp", bufs=18))
            normp = sctx.enter_context(tc.tile_pool(name="norm", bufs=2))
            ps_proj = sctx.enter_context(
                tc.tile_pool(name="ps_proj", bufs=2, space="PSUM"))
            ps_sc = sctx.enter_context(
                tc.tile_pool(name="ps_sc", bufs=2, space="PSUM"))
            dscr = sctx.enter_context(
                tc.tile_pool(name="dscr", bufs=8, space="DRAM"))

            qts, kts = {}, {}

            def emit_qk(hp):
                qt = qk8p.tile([128, S], bf16, tag="qt", name=f"qt{hp}")
                kt = qk8p.tile([128, S], bf16, tag="kt", name=f"kt{hp}")
                qts[hp], kts[hp] = qt, kt
                for dst, w, b in ((qt, wq, bqs), (kt, wk, bks)):
                    for sc in range(2):
                        sl = slice(sc * 512, (sc + 1) * 512)
                        ps = ps_proj.tile([128, 512], f32, tag="pp",
                                          name=f"pp{hp}_{dst.name}{sc}")
                        for k in range(NE):
                            nc.tensor.matmul(
                                ps,
                                lhsT=w[k][:, hp * 128:(hp + 1) * 128],
                                rhs=xt[k][:, sl],
                                start=(k == 0), stop=(k == NE - 1),
                            )
                        nc.vector.tensor_scalar_add(dst[:, sl], ps, b[hp])

            def emit_v():
                # bv is folded into the output bias on the host
                # (bo' = bo + Wo @ bv); the denominator ones-columns are
                # memset directly, so no augmented k-step is needed.
                with tc.tile_pool(name="ps_v", bufs=1, space="PSUM") as ps_v:
                    for i in range(NT):
                        ps = ps_v.tile([128, VW], f32, tag="pv", name=f"pv{i}")
                        for k in range(NE):
                            for off, sz in ((0, 512), (512, VW - 512)):
                                nc.tensor.matmul(
                                    ps[:, off:off + sz],
                                    lhsT=xa(k)[:, i * 128:(i + 1) * 128],
                                    rhs=wv[k][:, off:off + sz],
                                    start=(k == 0), stop=(k == NE - 1),
                                )
                        nc.vector.tensor_copy(vt[i], ps)
                        c0 = vt[i][:, DH:DH + 1]
                        ones_cols = bass.AP(tensor=c0.tensor, offset=c0.offset,
                                            ap=[list(c0.ap[0]), [HW, H]])
                        nc.vector.memset(ones_cols, 1.0)

            emit_qk(0)

            for hp in range(H // 2):
                qt, kt = qts.pop(hp), kts.pop(hp)
                exps = [[], []]
                for t in range(NT):
                    for half in range(2):
                        hb = half * 64
                        ps = ps_sc.tile([128, 1024], f32, tag="sc",
                                        name=f"sc{hp}_{t}_{half}")
                        for sc in range(2):
                            nc.tensor.matmul(
                                ps[:, sc * 512:(sc + 1) * 512],
                                lhsT=kt[hb:hb + 64, t * 128:(t + 1) * 128],
                                rhs=qt[hb:hb + 64, sc * 512:(sc + 1) * 512],
                                start=True, stop=True,
                                tile_position=(hb, 0),
                            )
                        ex = expp.tile([128, 1024], bf16, tag="e",
                                       name=f"e{hp}_{t}_{half}")
                        nc.scalar.activation(ex, ps, EXP, scale=0.125)
                        exps[half].append(ex)
                if hp == 0:
                    emit_v()
                    # V psum banks freed; attention psum takes their place.
                    ps_at = sctx.enter_context(
                        tc.tile_pool(name="ps_at", bufs=2, space="PSUM"))
                if hp + 1 < H // 2:
                    emit_qk(hp + 1)
                last = hp == H // 2 - 1
                if not last:
                    for half in range(2):
                        head = hp * 2 + half
                        asb = normp.tile([HW, 1024], f32, tag="asb",
                                         name=f"asb{head}")
                        for sc in range(2):
                            pa = ps_at.tile([HW, 512], f32, tag="at",
                                            name=f"at{head}_{sc}")
                            for t in range(NT):
                                nc.tensor.matmul(
                                    pa,
                                    lhsT=vt[t][:, head * HW:(head + 1) * HW],
                                    rhs=exps[half][t][:,
                                                      sc * 512:(sc + 1) * 512],
                                    start=(t == 0), stop=(t == NT - 1),
                                )
                            nc.vector.tensor_copy(
                                asb[:, sc * 512:(sc + 1) * 512], pa)
                        dn1 = dscr.tile([1, 1024], f32, tag="d1",
                                        name=f"dn1{head}")
                        nc.gpsimd.dma_start(out=dn1, in_=asb[64:65, :])
                        den8 = normp.tile([128, 8], f32, tag="d8",
                                          name=f"den8{head}")
                        dn1_r = bass.AP(tensor=dn1.tensor, offset=dn1.offset,
                                        ap=[[8, 128], [1, 8]])
                        nc.gpsimd.dma_start(out=den8, in_=dn1_r)
                        rcp8 = normp.tile([128, 8], f32, tag="r8",
                                          name=f"rcp8{head}")
                        nc.vector.reciprocal(rcp8, den8)
                        dn2 = dscr.tile([1, 1024], f32, tag="d2",
                                        name=f"dn2{head}")
                        dn2_w = bass.AP(tensor=dn2.tensor, offset=dn2.offset,
                                        ap=[[8, 128], [1, 8]])
                        nc.gpsimd.dma_start(out=dn2_w, in_=rcp8)
                        rcb = normp.tile([64, 1024], f32, tag="rcb",
                                         name=f"rcb{head}")
                        nc.gpsimd.dma_start(
                            out=rcb, in_=dn2[0].partition_broadcast(64))
                        nc.gpsimd.tensor_mul(
                            catt[hp][half * 64:(half + 1) * 64, :],
                            asb[0:64, :], rcb)
                else:
                    # Last pair: attention split into column-half groups so
                    # the first half's denominator (and its normalize chain)
                    # completes ~6us before the full attention does.  Heads
                    # interleave per column so both chains launch early, on
                    # separate DMA queues.
                    asbs = [normp.tile([HW, 1024], f32, tag="asb",
                                       name=f"asbL{h}") for h in range(2)]
                    rcbs = [[None, None], [None, None]]
                    for cc in range(2):
                        cs = slice(cc * 512, (cc + 1) * 512)
                        for half in range(2):
                            head = hp * 2 + half
                            pa = ps_at.tile([HW, 512], f32, tag="at",
                                            name=f"atL{head}_{cc}")
                            for t in range(NT):
                                nc.tensor.matmul(
                                    pa,
                                    lhsT=vt[t][:, head * HW:(head + 1) * HW],
                                    rhs=exps[half][t][:, cs],
                                    start=(t == 0), stop=(t == NT - 1),
                                )
                            dq = nc.scalar if half == 1 else nc.gpsimd
                            nc.vector.tensor_copy(asbs[half][:, cs], pa)
                            dn1 = dscr.tile([1, 1024], f32, tag=f"dL{cc}",
                                            name=f"dnL{head}_{cc}")
                            dq.dma_start(out=dn1[:, 0:512],
                                         in_=asbs[half][64:65, cs])
                            den8 = normp.tile([64, 8], f32, tag=f"d8L{cc}",
                                              name=f"d8L{head}_{cc}")
                            dn1_r = bass.AP(tensor=dn1.tensor,
                                            offset=dn1.offset,
                                            ap=[[8, 64], [1, 8]])
                            dq.dma_start(out=den8, in_=dn1_r)
                            rcp8 = normp.tile([64, 8], f32, tag=f"r8L{cc}",
                                              name=f"r8L{head}_{cc}")
                            nc.vector.reciprocal(rcp8, den8)
                            dn2 = dscr.tile([1, 1024], f32, tag=f"eL{cc}",
                                            name=f"dn2L{head}_{cc}")
                            dn2_w = bass.AP(tensor=dn2.tensor,
                                            offset=dn2.offset,
                                            ap=[[8, 64], [1, 8]])
                            dq.dma_start(out=dn2_w, in_=rcp8)
                            rcb = normp.tile([64, 512], f32, tag=f"rbL{cc}",
                                             name=f"rbL{head}_{cc}")
                            dq.dma_start(
                                out=rcb,
                                in_=dn2[0, 0:512].partition_broadcast(64))
                            rcbs[half][cc] = rcb
                        for half in range(2):
                            nc.vector.tensor_mul(
                                catt[hp][half * 64:(half + 1) * 64, cs],
                                asbs[half][0:64, cs], rcbs[half][cc])

        # ---- P4: output projection ----
        # Two waves of 4 m-tiles; each psum group is paused after the
        # catt[0..4]+bias part (ready as soon as pair 4 is done, fills the
        # last pair's normalize latency) and finished with the k=5 matmuls
        # once catt[5] lands.  osb tiles live in `singles` so no pool-open
        # alias barrier gates the PE.
        def ca(k):
            return catt[k] if k < NE else ones_row

        osb_t = [singles.tile([128, E], f32, tag=f"osb{i}", name=f"osb{i}")
                 for i in range(NT)]
        with tc.tile_pool(name="ps_o", bufs=4, space="PSUM") as ps_o:
            KL = [0, 1, 2, 3, 4, 5]
            for m in range(NT):
                ps = ps_o.tile([128, E], f32, tag="po", name=f"po{m}")
                for k in KL:
                    for off, sz in ((0, 512), (512, E - 512)):
                        nc.tensor.matmul(
                            ps[:, off:off + sz],
                            lhsT=ca(k)[:, m * 128:(m + 1) * 128],
                            rhs=wo[k][:, off:off + sz],
                            start=(k == KL[0]), stop=(k == KL[-1]),
                        )
                ot = osb_t[m]
                # bias add fused into the psum drain (bo pre-broadcast on
                # host) instead of a ones-row matmul k-step
                nc.vector.tensor_add(ot, ps, bo_bc)
                nc.sync.dma_start(out=out_d[m * 128:(m + 1) * 128, :],
                                  in_=ot)

    _dedupe_ldweights(nc)
    if split_waits:
        _split_multiwaits(nc)
    return nc


def _tile_img(Wt):
    """[E, width] -> SBUF image [128, NE*width] (row p = k-tile rows p)."""
    width = Wt.shape[1]
    return np.ascontiguousarray(
        Wt.reshape(NE, 128, width).transpose(1, 0, 2).reshape(128, NE * width))


def _prep_weights(Wq, bq, Wk, bk, Wv, bv, Wo, bo):
    bf16 = ml_dtypes.bfloat16

    wqt = _tile_img(np.asarray(Wq, np.float32).reshape(H * DH, E).T).astype(bf16)
    wkt = _tile_img(np.asarray(Wk, np.float32).reshape(H * DH, E).T).astype(bf16)
    bqv = np.ascontiguousarray(
        np.asarray(bq, np.float32).reshape(NE, 128).T).astype(np.float32)
    bkv = np.ascontiguousarray(
        np.asarray(bk, np.float32).reshape(NE, 128).T).astype(np.float32)

    wvt = np.zeros((E, VW), np.float32)
    Wv = np.asarray(Wv, np.float32)
    bv = np.asarray(bv, np.float32)
    for h in range(H):
        wvt[:, h * HW:h * HW + DH] = Wv[h].T
    wvt = _tile_img(wvt).astype(bf16)

    Wo = np.asarray(Wo, np.float32)
    bo = np.asarray(bo, np.float32)
    wot = _tile_img(Wo.T).astype(bf16)
    # bv contributes a constant Wo @ bv to every output row; fold into bo
    bo_eff = bo + Wo @ bv.reshape(E)
    bob = np.ascontiguousarray(
        np.broadcast_to(bo_eff.reshape(1, E), (128, E))).astype(bf16)
    return wqt, wkt, bqv, bkv, wvt, wot, bob


def _install_ntff_shim():
    """Provide antenv.axon_hooks (absent in this image) so trace=True can
    drive NRT profiling through libaxon_pjrt.so.  Dev-only; harmless no-op
    when anything is missing."""
    import sys, types
    try:
        import antenv.axon_hooks  # noqa
        return
    except ImportError:
        pass
    try:
        import antenv
        mod = types.ModuleType("antenv.axon_hooks")
        _state = {}
        mod.set_axon_ntff_profile_hook = lambda h: _state.update(h=h)
        mod.get_axon_ntff_profile_hook = lambda: _state.get("h")
        sys.modules["antenv.axon_hooks"] = mod
        antenv.axon_hooks = mod
        from trn_agent_boot.trn_boot import _ntff_profile_via_ctypes
        hook = _ntff_profile_via_ctypes("/opt/axon/libaxon_pjrt.so")
        if hook is not None:
            mod.set_axon_ntff_profile_hook(hook)
    except Exception as e:  # pragma: no cover
        print(f"ntff shim failed: {e}")


def kernel(x, Wq, bq, Wk, bk, Wv, bv, Wo, bo):
    from concourse.bass_utils import run_bass_kernel_spmd

    if "nc" not in _cache:
        _cache["nc"] = _build_bass()
    nc = _cache["nc"]

    wqt, wkt, bqv, bkv, wvt, wot, bob = _prep_weights(
        Wq, bq, Wk, bk, Wv, bv, Wo, bo)
    x = np.asarray(x, np.float32)
    in_maps = [
        {"xtr": _tile_img(np.ascontiguousarray(x[b].T)
                          ).astype(ml_dtypes.bfloat16),
         "wqt": wqt, "wkt": wkt, "bq": bqv, "bk": bkv,
         "wvt": wvt, "wot": wot, "bob": bob}
        for b in range(B)
    ]
    trace = bool(int(os.environ.get("MHA_TRACE", "0")))
    if trace:
        _install_ntff_shim()
    if int(os.environ.get("MHA_LDWOPT", "0")):
        _patch_ldw_opt()
    res = run_bass_kernel_spmd(nc, in_maps, list(range(B)), trace=trace)
    _cache["last_results"] = res
    return np.stack([res.results[b]["out"] for b in range(B)]).astype(np.float32)
